# revision 11
# baseline (speedup 1.0000x reference)
"""AutoCorrelation (Autoformer-style) forward on 8 Trainium2 NeuronCores.

kernel(**inputs) takes FULL unsharded inputs, returns the FULL (B, L, D) output.

Sharding: 32 (batch, head) pairs split 4-per-core (cores 0-3 batch 0, cores 4-7
batch 1). The ENTIRE pipeline runs on device per core: Q/K/V projections
(fp16 operands, fp32 PSUM), circular autocorrelation via Q@K^T accumulated
into rotated PSUM windows, a 3-stage shear (indirect_copy per 16-partition
group + permutation matmuls between stages) and a ones-matmul partition
reduction to get corr[d], top-8 delay selection (max_with_indices), softmax,
weighted circular gather of V (indirect_copy), and the output projection
(row-sharded; partials summed on host with bo).

Hardcoded shapes: B=2, L=4096, D=1024, H=16, Dk=64, top_k=8.
Self-contained: reads nothing from /root/problem.
"""

import math
import sys

import numpy as np

if "/opt/trn_rl_repo" not in sys.path:
    sys.path.insert(0, "/opt/trn_rl_repo")

B = 2
L = 4096
D_MODEL = 1024
NHEAD = 16
DK = D_MODEL // NHEAD  # 64
TOP_K = min(max(1, int(math.log(L + 1))), L)  # 8
N_CORES = 8
HPC = 4  # heads per core
COLS = HPC * DK  # 256 projection columns per core


# ---------------------------------------------------------------------------
# host fallback (numerically exact, slow) — used if the device path fails
# ---------------------------------------------------------------------------
def _tail_host(Q, K, V, Wo, bo):
    Qf = np.fft.rfft(Q, axis=2)
    Kf = np.fft.rfft(K, axis=2)
    corr = np.fft.irfft(Qf * np.conj(Kf), n=L, axis=2)
    corr_mean = corr.mean(axis=-1).astype(np.float32)

    idx = np.argsort(-corr_mean, axis=-1, kind="stable")[..., :TOP_K]
    w = np.take_along_axis(corr_mean, idx, axis=-1)
    w = np.exp(w - w.max(axis=-1, keepdims=True))
    w = w / w.sum(axis=-1, keepdims=True)

    out = np.zeros((B, NHEAD, L, DK), dtype=np.float32)
    ar = np.arange(L)
    for b in range(B):
        for h in range(NHEAD):
            acc = np.zeros((L, DK), dtype=np.float32)
            for t in range(TOP_K):
                acc += w[b, h, t] * V[b, h][(ar + int(idx[b, h, t])) % L]
            out[b, h] = acc

    out = out.transpose(0, 2, 1, 3).reshape(B * L, D_MODEL)
    return (out @ Wo + bo).reshape(B, L, D_MODEL).astype(np.float32)


def _forward_host(query, key, value, Wq, bq, Wk, bk, Wv, bv, Wo, bo):
    def proj(x, W, b):
        p = (x.reshape(B * L, D_MODEL) @ W + b).astype(np.float32)
        return p.reshape(B, L, NHEAD, DK).transpose(0, 2, 1, 3)

    return _tail_host(proj(query, Wq, bq), proj(key, Wk, bk), proj(value, Wv, bv), Wo, bo)


# ---------------------------------------------------------------------------
# device kernel
# ---------------------------------------------------------------------------
_NC_CACHE = {}


def _const_tables():
    """Constant tables for the on-device shear/gather.

    Shear: corr[d] = sum_p racc[p, (p - d) % L]; per-row shift p = 16g + 2w1
    + w0 applied via three per-16-group indirect_copy stages with row
    permutations (matmuls) between: A shifts 16g (+ index reversal), then
    rows regroup by w1 (pi1), B shifts 2*w1, regroup by w0 (pi2), C shifts w0.
    """

    def ic_table(lists):
        t = np.zeros((128, 256), np.uint16)
        for g in range(8):
            t[16 * g : 16 * (g + 1), :] = lists[g].reshape(256, 16).T
        return t

    taba = ic_table([(16 * g - np.arange(L)) % L for g in range(8)])
    tabb = ic_table([(np.arange(L) - 2 * g) % L for g in range(8)])
    tabc = ic_table([(np.arange(L) - g // 4) % L for g in range(8)])

    gtbl = np.zeros((128, 256), np.uint16)
    for p in range(128):
        gtbl[p] = 16 * np.arange(256) + (p % 16)

    pi1 = np.zeros(128, np.int64)
    for p in range(128):
        g, w = p // 16, p % 16
        pi1[p] = 16 * (w // 2) + 2 * g + (w % 2)
    pi2 = np.array([64 * (p % 2) + p // 2 for p in range(128)])
    pm1 = np.zeros((128, 128), np.float16)
    pm1[np.arange(128), pi1] = 1.0
    pm2 = np.zeros((128, 128), np.float16)
    pm2[np.arange(128), pi2] = 1.0
    return taba, tabb, tabc, gtbl, pm1, pm2


def _build_nc():
    import concourse.bacc as bacc
    import concourse.mybir as mybir
    from concourse.tile import TileContext

    f32 = mybir.dt.float32
    f16 = mybir.dt.float16
    u16 = mybir.dt.uint16
    u32 = mybir.dt.uint32

    nc = bacc.Bacc(None, target_bir_lowering=False, dynamic_dma_scratch_size=2048)

    ins = {}
    for nm in ("xq", "xk", "xv"):
        ins[nm] = nc.declare_dram_parameter(nm, [D_MODEL, L], f16, isOutput=False)
    for nm in ("wq", "wk", "wv"):
        ins[nm] = nc.declare_dram_parameter(nm, [D_MODEL, COLS], f16, isOutput=False)
    ins["wo"] = nc.declare_dram_parameter("wo", [COLS, D_MODEL], f16, isOutput=False)
    for nm in ("taba", "tabb", "tabc", "gtbl"):
        ins[nm] = nc.declare_dram_parameter(nm, [128, 256], u16, isOutput=False)
    for nm in ("pm1", "pm2"):
        ins[nm] = nc.declare_dram_parameter(nm, [128, 128], f16, isOutput=False)

    out_d = nc.declare_dram_parameter("out", [L, D_MODEL], f16, isOutput=True)
    corr_d = nc.declare_dram_parameter("corr", [HPC, L], f32, isOutput=True)
    ti_d = nc.declare_dram_parameter("ti", [HPC, 8], u32, isOutput=True)
    w8_d = nc.declare_dram_parameter("w8", [HPC, 8], f32, isOutput=True)

    KT = D_MODEL // 128  # 8 contraction chunks
    NW = L // 512  # 8 t-windows

    with TileContext(nc) as tc:
        with (
            tc.tile_pool(name="wp", bufs=1) as wp,
            tc.tile_pool(name="xs", bufs=2) as xs,
            tc.tile_pool(name="qkv", bufs=1) as qkv,
            tc.tile_pool(name="shp", bufs=1) as shp,
            tc.tile_pool(name="gp", bufs=1) as gp,
            tc.tile_pool(name="sm", bufs=1) as sm,
            tc.tile_pool(name="oev", bufs=2) as oevp,
            tc.tile_pool(name="pp", bufs=1, space="PSUM") as pp,
        ):
            P8 = pp.tile([128, L], f32, tag="P8")

            # ---- constants
            tabs = {}
            for nm in ("taba", "tabb", "tabc", "gtbl"):
                t = sm.tile([128, 256], u16, tag=nm, name=nm)
                nc.sync.dma_start(out=t[:, :], in_=ins[nm][:, :])
                tabs[nm] = t
            pms = {}
            for nm in ("pm1", "pm2"):
                t = sm.tile([128, 128], f16, tag=nm, name=nm)
                nc.sync.dma_start(out=t[:, :], in_=ins[nm][:, :])
                pms[nm] = t
            ones16 = sm.tile([128, 1], f16, tag="ones16")
            nc.vector.memset(ones16[:, :], 1.0)
            gtblf_t = sm.tile([128, 256], f32, tag="gtblf")
            nc.vector.tensor_copy(gtblf_t[:, :], tabs["gtbl"][:, :])

            # ---- weights
            wt = {}
            for nm in ("wq", "wk", "wv"):
                t = wp.tile([128, KT * COLS], f16, tag=nm, name=nm)
                for kc in range(KT):
                    nc.sync.dma_start(
                        out=t[:, kc * COLS : (kc + 1) * COLS],
                        in_=ins[nm][kc * 128 : (kc + 1) * 128, :],
                    )
                wt[nm] = t
            wo_t = []
            for q in range(2):
                t = wp.tile([128, D_MODEL], f16, tag=f"wo{q}", name=f"wo{q}")
                nc.sync.dma_start(out=t[:, :], in_=ins["wo"][q * 128 : (q + 1) * 128, :])
                wo_t.append(t)

            # ---- projections -> QT/KT/VT pair tiles [128ch x L] f16
            proj = {"q": [], "k": [], "v": []}
            for key_ in ("q", "k", "v"):
                for q in range(2):
                    proj[key_].append(
                        qkv.tile([128, L], f16, tag=f"{key_}t{q}", name=f"{key_}t{q}")
                    )
            pswin = 0
            for key_, xnm, wnm in (("q", "xq", "wq"), ("k", "xk", "wk"), ("v", "xv", "wv")):
                for n in range(NW):
                    xst = xs.tile([128, KT * 512], f16, tag="xst")
                    for kc in range(KT):
                        nc.sync.dma_start(
                            out=xst[:, kc * 512 : (kc + 1) * 512],
                            in_=ins[xnm][kc * 128 : (kc + 1) * 128, n * 512 : (n + 1) * 512],
                        )
                    for m in range(2):
                        win = (pswin % 4) * 512
                        pswin += 1
                        for kc in range(KT):
                            nc.tensor.matmul(
                                P8[:, win : win + 512],
                                wt[wnm][:, kc * COLS + m * 128 : kc * COLS + (m + 1) * 128],
                                xst[:, kc * 512 : (kc + 1) * 512],
                                start=(kc == 0),
                                stop=(kc == KT - 1),
                                skip_group_check=True,
                            )
                        nc.vector.tensor_copy(
                            proj[key_][m][:, n * 512 : (n + 1) * 512], P8[:, win : win + 512]
                        )

            # ---- per-head racc: S''=Q@K^T accumulated at e=(s-128i)%L
            a1s = []
            for h in range(HPC):
                pair, half = h // 2, h % 2
                rows = slice(64 * half, 64 * half + 64)
                qt, kt = proj["q"][pair], proj["k"][pair]
                for i in range(32):
                    lhs = qt[rows, 128 * i : 128 * (i + 1)]
                    r = (-128 * i) % 512
                    for bblk in range(NW):
                        e0 = (512 * bblk - 128 * i) % L
                        pieces = (
                            [(e0, 512, 0)]
                            if r == 0
                            else [(e0, 512 - r, 0), ((e0 + 512 - r) % L, r, 512 - r)]
                        )
                        for pe, plen, soff in pieces:
                            nc.tensor.matmul(
                                P8[:, pe : pe + plen],
                                lhs,
                                kt[rows, 512 * bblk + soff : 512 * bblk + soff + plen],
                                start=(i == 0),
                                stop=(i == 31),
                                skip_group_check=True,
                            )
                # evict fp32 PSUM -> fp16 SBUF, then shear stage A (shift 16g + reversal)
                a0 = shp.tile([128, L], f16, tag="a0", bufs=2)
                nc.vector.tensor_copy(a0[:, :], P8[:, :])
                a1 = shp.tile([128, L], f16, tag="a1", bufs=4)
                for cc in range(4):
                    nc.gpsimd.indirect_copy(
                        a1[:, 1024 * cc : 1024 * (cc + 1)],
                        a0[:, :],
                        tabs["taba"][:, 64 * cc : 64 * (cc + 1)],
                        True,
                    )
                a1s.append(a1)

            # ---- tail per head: perm1 -> B -> perm2 -> C -> ones reduction
            tif_t, w8_t = [], []
            for h in range(HPC):
                b0 = shp.tile([128, L], f16, tag="sE", bufs=2, name="b0")
                for j in range(NW):
                    win = 512 * j
                    nc.tensor.matmul(
                        P8[:, win : win + 512], pms["pm1"][:, :],
                        a1s[h][:, win : win + 512], start=True, stop=True,
                        skip_group_check=True,
                    )
                    nc.vector.tensor_copy(b0[:, win : win + 512], P8[:, win : win + 512])
                b1 = shp.tile([128, L], f16, tag="sF", bufs=2, name="b1")
                for cc in range(4):
                    nc.gpsimd.indirect_copy(
                        b1[:, 1024 * cc : 1024 * (cc + 1)], b0[:, :],
                        tabs["tabb"][:, 64 * cc : 64 * (cc + 1)], True,
                    )
                c0 = shp.tile([128, L], f16, tag="sE", bufs=2, name="c0")
                for j in range(NW):
                    win = 512 * j
                    nc.tensor.matmul(
                        P8[:, win : win + 512], pms["pm2"][:, :],
                        b1[:, win : win + 512], start=True, stop=True,
                        skip_group_check=True,
                    )
                    nc.vector.tensor_copy(c0[:, win : win + 512], P8[:, win : win + 512])
                c1 = shp.tile([128, L], f16, tag="sF", bufs=2, name="c1")
                for cc in range(4):
                    nc.gpsimd.indirect_copy(
                        c1[:, 1024 * cc : 1024 * (cc + 1)], c0[:, :],
                        tabs["tabc"][:, 64 * cc : 64 * (cc + 1)], True,
                    )
                for j in range(NW):
                    win = 512 * j
                    nc.tensor.matmul(
                        P8[0:1, win : win + 512], ones16[:, :],
                        c1[:, win : win + 512], start=True, stop=True,
                        skip_group_check=True,
                    )
                co = sm.tile([1, L], f32, tag="corr")
                nc.vector.tensor_copy(co[:, :], P8[0:1, :])
                nc.sync.dma_start(out=corr_d[h : h + 1, :], in_=co[:, :])

                tv = sm.tile([1, 8], f32, tag=f"tv{h}", name=f"tv{h}")
                ti = sm.tile([1, 8], u32, tag=f"ti{h}", name=f"ti{h}")
                nc.vector.max_with_indices(tv[:, :], ti[:, :], co[:, :])
                nc.sync.dma_start(out=ti_d[h : h + 1, :], in_=ti[:, :])
                tif = sm.tile([1, 8], f32, tag=f"tif{h}", name=f"tif{h}")
                nc.vector.tensor_copy(tif[:, :], ti[:, :])
                tif_t.append(tif)

                negmax = sm.tile([1, 1], f32, tag=f"nm{h}", name=f"nm{h}")
                nc.vector.tensor_scalar_mul(negmax[:, :], tv[:, 0:1], -1.0)
                e8 = sm.tile([1, 8], f32, tag=f"e8{h}", name=f"e8{h}")
                nc.scalar.activation(
                    e8[:, :], tv[:, :], mybir.ActivationFunctionType.Exp,
                    bias=negmax[:, 0:1], scale=1.0,
                )
                ssum = sm.tile([1, 1], f32, tag=f"ss{h}", name=f"ss{h}")
                nc.vector.tensor_reduce(
                    ssum[:, :], e8[:, :], mybir.AxisListType.X, mybir.AluOpType.add
                )
                rs = sm.tile([1, 1], f32, tag=f"rs{h}", name=f"rs{h}")
                nc.vector.reciprocal(rs[:, :], ssum[:, :])
                w8 = sm.tile([1, 8], f32, tag=f"w8{h}", name=f"w8{h}")
                nc.vector.tensor_scalar_mul(w8[:, :], e8[:, :], rs[:, 0:1])
                w8_t.append(w8)
                nc.sync.dma_start(out=w8_d[h : h + 1, :], in_=w8[:, :])

            # ---- V-combine per pair: vw[p,t] = sum_k w_k * VT[p, (t+d_k)%L]
            vw = []
            for q in range(2):
                hA, hB = 2 * q, 2 * q + 1
                dcol = sm.tile([128, 8], f32, tag=f"dcol{q}", name=f"dcol{q}")
                nc.gpsimd.partition_broadcast(dcol[0:64, :], tif_t[hA][:, :], channels=64)
                nc.gpsimd.partition_broadcast(dcol[64:128, :], tif_t[hB][:, :], channels=64)
                wcol = sm.tile([128, 8], f32, tag=f"wcol{q}", name=f"wcol{q}")
                nc.gpsimd.partition_broadcast(wcol[0:64, :], w8_t[hA][:, :], channels=64)
                nc.gpsimd.partition_broadcast(wcol[64:128, :], w8_t[hB][:, :], channels=64)

                vwt = gp.tile([128, L], f16, tag=f"vw{q}", bufs=1, name=f"vw{q}")
                nc.vector.memset(vwt[:, :], 0.0)
                for k in range(TOP_K):
                    idxf = gp.tile([128, 256], f32, tag="idxf", bufs=1)
                    nc.vector.tensor_scalar(
                        idxf[:, :], gtblf_t[:, :], dcol[:, k : k + 1], None,
                        mybir.AluOpType.add,
                    )
                    ge = gp.tile([128, 256], f32, tag="ge", bufs=1)
                    nc.vector.tensor_scalar(
                        ge[:, :], idxf[:, :], 4096.0, None, mybir.AluOpType.is_ge
                    )
                    nc.vector.scalar_tensor_tensor(
                        idxf[:, :], ge[:, :], -4096.0, idxf[:, :],
                        mybir.AluOpType.mult, mybir.AluOpType.add,
                    )
                    idxk = gp.tile([128, 256], u16, tag="idxk", bufs=1)
                    nc.vector.tensor_copy(idxk[:, :], idxf[:, :])
                    gk = gp.tile([128, L], f16, tag="gk", bufs=2)
                    for cc in range(4):
                        nc.gpsimd.indirect_copy(
                            gk[:, 1024 * cc : 1024 * (cc + 1)],
                            proj["v"][q][:, :],
                            idxk[:, 64 * cc : 64 * (cc + 1)],
                            True,
                        )
                    nc.vector.scalar_tensor_tensor(
                        vwt[:, :], gk[:, :], wcol[:, k : k + 1], vwt[:, :],
                        mybir.AluOpType.mult, mybir.AluOpType.add,
                    )
                vw.append(vwt)

            # ---- output projection: out[t,:] = sum_h vw_h[t,:] @ wo_h
            pswin = 0
            for j in range(32):
                for nn in range(2):
                    win = (pswin % 4) * 512
                    pswin += 1
                    for h in range(HPC):
                        pair, half = h // 2, h % 2
                        rows = slice(64 * half, 64 * half + 64)
                        nc.tensor.matmul(
                            P8[:, win : win + 512],
                            vw[pair][rows, 128 * j : 128 * (j + 1)],
                            wo_t[pair][rows, 512 * nn : 512 * (nn + 1)],
                            start=(h == 0),
                            stop=(h == HPC - 1),
                            skip_group_check=True,
                        )
                    ot = oevp.tile([128, 512], f16, tag="ot")
                    nc.vector.tensor_copy(ot[:, :], P8[:, win : win + 512])
                    nc.sync.dma_start(
                        out=out_d[128 * j : 128 * (j + 1), 512 * nn : 512 * (nn + 1)],
                        in_=ot[:, :],
                    )

    nc.finalize()
    return nc


def _get_nc():
    if "nc" not in _NC_CACHE:
        _NC_CACHE["nc"] = _build_nc()
    return _NC_CACHE["nc"]


def _core_inputs(query, key, value, Wq, Wk, Wv, Wo):
    taba, tabb, tabc, gtbl, pm1, pm2 = _const_tables()
    f16 = np.float16
    xT = {}
    for b in range(B):
        xT[("q", b)] = np.ascontiguousarray(query[b].T, dtype=f16)
        xT[("k", b)] = np.ascontiguousarray(key[b].T, dtype=f16)
        xT[("v", b)] = np.ascontiguousarray(value[b].T, dtype=f16)
    in_maps = []
    for c in range(N_CORES):
        b = c // 4
        h0 = (c % 4) * HPC
        cols = slice(h0 * DK, h0 * DK + COLS)
        in_maps.append({
            "xq": xT[("q", b)],
            "xk": xT[("k", b)],
            "xv": xT[("v", b)],
            "wq": Wq[:, cols].astype(f16),
            "wk": (Wk[:, cols] / DK).astype(f16),  # fold corr mean(1/Dk) into K
            "wv": Wv[:, cols].astype(f16),
            "wo": Wo[h0 * DK : h0 * DK + COLS, :].astype(f16),
            "taba": taba, "tabb": tabb, "tabc": tabc, "gtbl": gtbl,
            "pm1": pm1, "pm2": pm2,
        })
    return in_maps


def _forward_device(query, key, value, Wq, bq, Wk, bk, Wv, bv, Wo, bo, spmd_kwargs=None):
    from concourse.bass_utils import run_bass_kernel_spmd

    nc = _get_nc()
    in_maps = _core_inputs(query, key, value, Wq, Wk, Wv, Wo)
    res = run_bass_kernel_spmd(nc, in_maps, list(range(N_CORES)), **(spmd_kwargs or {}))

    out = np.zeros((B, L, D_MODEL), dtype=np.float32)
    for c in range(N_CORES):
        out[c // 4] += np.asarray(res.results[c]["out"], dtype=np.float32)
    out += bo.astype(np.float32)
    return out, res


def kernel(**inputs):
    inputs = {k: np.asarray(v, dtype=np.float32) for k, v in inputs.items()}
    if any(np.any(inputs[k]) for k in ("bq", "bk", "bv")):
        return _forward_host(**inputs)
    try:
        out, _ = _forward_device(**inputs)
        return out
    except Exception:
        import traceback

        traceback.print_exc()
        return _forward_host(**inputs)
